# revision 14
# baseline (speedup 1.0000x reference)
"""Trainium2 Bass kernel for nn_BinaryXnorExceptOutliersLinearColumn.

Computes y = x @ w_bin^T + bias where w_bin rewrites the non-outlier columns
(by column L1-norm quantile band) of w as w * scale (scale = per-row mean of
|w| over the selected columns).

Decomposition used on device (per core, OUT rows sharded 8 ways):
    y = [x @ w^T + bias] + (scale - 1) * (xm @ w^T),   xm = x * mask
with the column mask derived from exact rank selection on near-exactly
summed column L1 norms (allreduce across cores), matching the reference's
jnp.quantile comparisons for this problem's data distribution.

Mask exactness strategy: column norms are accumulated per-core via
DVE free-dim reductions on exact fp32 transposed tiles, then combined
across cores exactly via a fixed-point hi/lo split AllReduce (hi parts are
integer-valued fp32 -> order-independent exact addition). Rank selection
(rank <= 204 or rank >= 3891) replicates the quantile band membership.
"""

import sys
import numpy as np

for _p in ("/opt/trn_rl_repo",):
    if _p not in sys.path:
        sys.path.insert(0, _p)

import concourse.bass as bass
import concourse.mybir as mybir
import concourse.tile as tile
import concourse.tile_utils as tile_utils
from concourse import bacc
from concourse.bass_utils import run_bass_kernel_spmd
from concourse.masks import make_identity
from concourse import bass_isa

F32 = mybir.dt.float32
F32R = mybir.dt.float32r
ALU = mybir.AluOpType
AX = mybir.AxisListType

N_CORES = 8
B, S, IN, OUT = 32, 8, 4096, 11008
TOK = B * S                      # 256
OSH = OUT // N_CORES             # 1376 out rows per core
KC = IN // 128                   # 32 k-chunks over IN
# free-dim groups over the per-core OUT shard (matmul N <= 512)
GROUPS = [(0, 512), (512, 512), (1024, 352)]
# rank selection boundaries (0-indexed order statistics), LOW_Q=0.05 HIGH_Q=0.95
#   modify = (rank <= 204) | (rank >= 3891)   [n=4096]
RANK_LO = 204
RANK_HI = 3891
GRID = 32768.0                   # 2^15 fixed-point scale for exact allreduce
RND = 8388608.0                  # 2^23 round-to-int constant

# allow using the full usable SBUF
tile_utils.max_sbuf_usage = 207 * 1024


def _tile_helper(pool):
    def _tile(shape, dtype, name):
        return pool.tile(shape, dtype, name=name, tag=name)
    return _tile


def _jblocks(goff, gsz):
    """128-row transpose blocks covering [goff, goff+gsz)."""
    out = []
    off = 0
    while off < gsz:
        p = min(128, gsz - off)
        out.append((off, p))
        off += p
    return out


def build():
    nc = bacc.Bacc("TRN2", target_bir_lowering=False, debug=False,
                   num_devices=N_CORES)

    w = nc.dram_tensor("w", [OSH, IN], F32, kind="ExternalInput")
    x = nc.dram_tensor("x", [TOK, IN], F32, kind="ExternalInput")
    bias = nc.dram_tensor("bias", [1, OSH], F32, kind="ExternalInput")
    sel = nc.dram_tensor("sel", [4, 128, KC], F32, kind="ExternalInput")
    y = nc.dram_tensor("y", [TOK, OSH], F32, kind="ExternalOutput")

    cc1_in = nc.dram_tensor("cc1_in", [2, 128, KC], F32)
    cc1_out = nc.dram_tensor("cc1_out", [2, 128, KC], F32, addr_space="Shared")
    cc2_in = nc.dram_tensor("cc2_in", [128, KC], F32)
    cc2_out = nc.dram_tensor("cc2_out", [128, KC], F32, addr_space="Shared")
    cn_lin = nc.dram_tensor("cn_lin", [IN], F32)
    y_tmp = nc.dram_tensor("y_tmp", [2, 128, OSH], F32)

    groups = list(range(N_CORES))

    with tile.TileContext(nc) as tc, \
         tc.tile_pool(name="resident", bufs=1) as resp:
        _tile = _tile_helper(resp)
        # ---------- constants / resident tensors ----------
        ident = _tile([128, 128], F32, name="ident")
        make_identity(nc, ident[:])
        ones1 = _tile([1, 128], F32, name="ones1")
        nc.vector.memset(ones1[:], 1.0)
        ones1r = _tile([1, 128], F32R, name="ones1r")
        nc.vector.tensor_copy(ones1r[:], ones1[:])
        ones128 = _tile([128, 1], F32, name="ones128")
        nc.vector.memset(ones128[:], 1.0)
        ones128r = _tile([128, 1], F32R, name="ones128r")
        nc.vector.tensor_copy(ones128r[:], ones128[:])

        wt_all = _tile([128, KC * OSH], F32R, name="wt_all")  # W^T resident
        cn_all = _tile([128, KC], F32, name="cn_all")
        mf_all = _tile([128, KC], F32, name="mf_all")
        mf_r = _tile([128, KC], F32R, name="mf_r")
        spb = _tile([128, OSH], F32, name="spb")          # (scale-1) bcast
        sm1r = _tile([1, OSH], F32R, name="sm1r")
        bias_r = _tile([1, OSH], F32R, name="bias_r")
        denom = _tile([1, 1], F32, name="denom")
        hs_all = _tile([128, KC], F32, name="hs_all")
        lo_all = _tile([128, KC], F32, name="lo_all")

        nc.gpsimd.dma_start(bias_r[:], bias[:])

        # ---------- stage 1: W load + PE transpose + colnorm partials ------
        with tc.tile_pool(name="s1_sbuf", bufs=2) as s1p, \
             tc.tile_pool(name="s1_psum", bufs=2, space="PSUM") as s1pp:
            for k in range(KC):
                wn = s1p.tile([128, 11 * 128], F32, tag="wn")
                # load the k-th column band of w: [OSH, 128]
                for j in range(11):
                    pj = min(128, OSH - j * 128)
                    nc.sync.dma_start(
                        wn[:pj, j * 128:j * 128 + 128],
                        w[j * 128:j * 128 + pj, k * 128:(k + 1) * 128])
                cnp = s1p.tile([128, len(GROUPS)], F32, tag="cnp")
                for g, (goff, gsz) in enumerate(GROUPS):
                    ps = s1pp.tile([128, 512], F32, tag="tr")
                    for (boff, pj) in _jblocks(goff, gsz):
                        j = (goff + boff) // 128
                        nc.tensor.transpose(
                            ps[:, boff:boff + pj],
                            wn[:pj, j * 128:j * 128 + 128],
                            ident[:pj, :pj])
                    # evacuate to resident W^T (rounds to f32r)
                    nc.scalar.copy(
                        wt_all[:, k * OSH + goff:k * OSH + goff + gsz],
                        ps[:, :gsz])
                    # exact fp32 |.| sum over this group of OUT columns
                    nc.vector.tensor_reduce(
                        cnp[:, g:g + 1], ps[:, :gsz], axis=AX.X, op=ALU.add,
                        apply_absolute_value=True)
                cnk = s1p.tile([128, 1], F32, tag="cnk")
                nc.vector.reduce_sum(cnk[:], cnp[:], axis=AX.X)
                # fixed-point hi/lo split for exact cross-core reduction
                u = s1p.tile([128, 1], F32, tag="u")
                t = s1p.tile([128, 1], F32, tag="t")
                gtt = s1p.tile([128, 1], F32, tag="gtt")
                nc.vector.tensor_scalar_mul(u[:], cnk[:], GRID)
                nc.vector.tensor_scalar_add(t[:], u[:], RND)
                nc.vector.tensor_scalar(
                    hs_all[:, k:k + 1], t[:], RND, None, op0=ALU.subtract)
                nc.vector.tensor_scalar_mul(gtt[:], hs_all[:, k:k + 1],
                                            1.0 / GRID)
                nc.vector.tensor_sub(lo_all[:, k:k + 1], cnk[:], gtt[:])

        # ---------- collective 1: exact colnorm allreduce ----------
        nc.sync.dma_start(cc1_in[0], hs_all[:])
        nc.sync.dma_start(cc1_in[1], lo_all[:])
        nc.gpsimd.collective_compute(
            "AllReduce", ALU.add, replica_groups=[groups],
            ins=[cc1_in[:]], outs=[cc1_out[:]])
        hs_sum = _tile([128, KC], F32, name="hs_sum")
        lo_sum = _tile([128, KC], F32, name="lo_sum")
        nc.sync.dma_start(hs_sum[:], cc1_out[0])
        nc.sync.dma_start(lo_sum[:], cc1_out[1])
        nc.vector.tensor_scalar_mul(cn_all[:], hs_sum[:], 1.0 / GRID)
        nc.vector.tensor_add(cn_all[:], cn_all[:], lo_sum[:])

        # roundtrip so cn is linear in DRAM, re-read in row chunks below
        nc.sync.dma_start(cn_lin.ap().rearrange("(k p) -> p k", p=128),
                          cn_all[:])
        cn_lin_row = cn_lin.ap().rearrange("(o n) -> o n", o=1)

        # ---------- stage 3: ranks for this core's 4 chunks + mask ---------
        with tc.tile_pool(name="s3_sbuf", bufs=1) as s3p, \
             tc.tile_pool(name="s3_psum", bufs=2, space="PSUM") as s3pp:
            sel_sb = s3p.tile([128, 4 * KC], F32, tag="sel")
            for tt in range(4):
                nc.sync.dma_start(sel_sb[:, tt * KC:(tt + 1) * KC], sel[tt])
            mask_contrib = s3p.tile([128, KC], F32, tag="mc")
            nc.vector.memset(mask_contrib[:], 0.0)
            cn_my = s3p.tile([128, 4], F32, tag="cnmy")
            tmp32 = s3p.tile([128, KC], F32, tag="tmp32")
            for tt in range(4):
                nc.vector.tensor_mul(tmp32[:], cn_all[:],
                                     sel_sb[:, tt * KC:(tt + 1) * KC])
                nc.vector.reduce_sum(cn_my[:, tt:tt + 1], tmp32[:], axis=AX.X)
            # broadcast cn to all partitions, in halves (exact fp32 matmul)
            rank = s3p.tile([128, 4], F32, tag="rank")
            racc = s3p.tile([128, 1], F32, tag="racc")
            vb = s3p.tile([128, 1024], F32, tag="vb")
            scr = s3p.tile([128, 1024], F32, tag="scr")
            for quarter in range(4):
                for cchunk in range(2):
                    off = quarter * 1024 + cchunk * 512
                    cr = s3p.tile([1, 512], F32, tag="cr", name="cr")
                    nc.sync.dma_start(cr[:], cn_lin_row[:, off:off + 512])
                    psb = s3pp.tile([128, 512], F32, tag="vbp")
                    nc.tensor.matmul(psb[:], ones1[:], cr[:],
                                     start=True, stop=True)
                    nc.vector.tensor_copy(
                        vb[:, cchunk * 512:(cchunk + 1) * 512], psb[:])
                for tt in range(4):
                    nc.vector.tensor_scalar(
                        scr[:], vb[:], cn_my[:, tt:tt + 1], 0.0,
                        op0=ALU.is_lt, op1=ALU.add, accum_out=racc[:])
                    if quarter == 0:
                        nc.vector.tensor_copy(rank[:, tt:tt + 1], racc[:])
                    else:
                        nc.vector.tensor_add(rank[:, tt:tt + 1],
                                             rank[:, tt:tt + 1], racc[:])
            # mask_my = (rank <= RANK_LO) | (rank >= RANK_HI)
            m1 = s3p.tile([128, 4], F32, tag="m1")
            m2 = s3p.tile([128, 4], F32, tag="m2")
            nc.vector.tensor_scalar(m1[:], rank[:], RANK_LO + 0.5, None,
                                    op0=ALU.is_lt)
            nc.vector.tensor_scalar(m2[:], rank[:], RANK_HI - 0.5, None,
                                    op0=ALU.is_gt)
            nc.vector.tensor_tensor(m1[:], m1[:], m2[:], op=ALU.logical_or)
            # scatter into the 32-chunk layout via sel columns
            for tt in range(4):
                nc.vector.tensor_scalar(
                    tmp32[:], sel_sb[:, tt * KC:(tt + 1) * KC],
                    m1[:, tt:tt + 1], None, op0=ALU.mult)
                nc.vector.tensor_add(mask_contrib[:], mask_contrib[:],
                                     tmp32[:])
            nc.sync.dma_start(cc2_in[:], mask_contrib[:])

        # ---------- collective 2: share masks ----------
        nc.gpsimd.collective_compute(
            "AllReduce", ALU.add, replica_groups=[groups],
            ins=[cc2_in[:]], outs=[cc2_out[:]])
        nc.sync.dma_start(mf_all[:], cc2_out[:])
        nc.vector.tensor_copy(mf_r[:], mf_all[:])

        # ---------- stage 4: C = x @ w^T + bias ----------
        with tc.tile_pool(name="s4_sbuf", bufs=3) as s4p, \
             tc.tile_pool(name="s4_psum", bufs=2, space="PSUM") as s4pp, \
             tc.tile_pool(name="s4C_psum", bufs=1, space="PSUM") as s4cp:
            psC = {}
            for m in range(2):
                for g, (goff, gsz) in enumerate(GROUPS):
                    psC[(m, g)] = s4cp.tile([128, gsz], F32, tag=f"C{m}{g}", name=f"psC{m}{g}")
                    nc.tensor.matmul(psC[(m, g)][:], ones1r[:],
                                     bias_r[:, goff:goff + gsz],
                                     start=True, stop=False)
            for k in range(KC):
                xn = s4p.tile([128, 256], F32, tag="xn")
                for m in range(2):
                    nc.sync.dma_start(
                        xn[:, m * 128:(m + 1) * 128],
                        x[m * 128:(m + 1) * 128, k * 128:(k + 1) * 128])
                xtp = s4pp.tile([128, 256], F32, tag="xtp")
                for m in range(2):
                    nc.tensor.transpose(xtp[:, m * 128:(m + 1) * 128],
                                        xn[:, m * 128:(m + 1) * 128],
                                        ident[:])
                xt = s4p.tile([128, 256], F32R, tag="xt")
                nc.scalar.copy(xt[:], xtp[:])
                for m in range(2):
                    for g, (goff, gsz) in enumerate(GROUPS):
                        nc.tensor.matmul(
                            psC[(m, g)][:],
                            xt[:, m * 128:(m + 1) * 128],
                            wt_all[:, k * OSH + goff:k * OSH + goff + gsz],
                            start=False, stop=(k == KC - 1))
            for m in range(2):
                for g, (goff, gsz) in enumerate(GROUPS):
                    stg = s4p.tile([128, 512], F32, tag="stg", name="stg")
                    nc.vector.tensor_copy(stg[:, :gsz], psC[(m, g)][:])
                    nc.sync.dma_start(y_tmp[m][:, goff:goff + gsz],
                                      stg[:, :gsz])

        # ---------- stage 5: scale ----------
        with tc.tile_pool(name="s5_sbuf", bufs=2) as s5p, \
             tc.tile_pool(name="s5_psum", bufs=1, space="PSUM") as s5pp:
            psS = [s5pp.tile([1, gsz], F32, tag=f"S{g}", name=f"psS{g}")
                   for g, (goff, gsz) in enumerate(GROUPS)]
            for k in range(KC):
                absk = s5p.tile([128, OSH], F32R, tag="absk")
                nc.scalar.activation(absk[:],
                                     wt_all[:, k * OSH:(k + 1) * OSH],
                                     mybir.ActivationFunctionType.Abs)
                for g, (goff, gsz) in enumerate(GROUPS):
                    nc.tensor.matmul(psS[g][:], mf_r[:, k:k + 1],
                                     absk[:, goff:goff + gsz],
                                     start=(k == 0), stop=(k == KC - 1))
            # denom = max(sum(mask), 1); invd = 1/denom
            mfs = s5p.tile([128, 1], F32, tag="mfs")
            nc.vector.reduce_sum(mfs[:], mf_all[:], axis=AX.X)
            par = s5p.tile([128, 1], F32, tag="par")
            nc.gpsimd.partition_all_reduce(par[:], mfs[:], 128,
                                           bass_isa.ReduceOp.add)
            nc.vector.tensor_scalar_max(denom[:], par[0:1, :], 1.0)
            invd = s5p.tile([1, 1], F32, tag="invd")
            nc.vector.reciprocal(invd[:], denom[:])
            smt = s5p.tile([1, 512], F32, tag="smt")
            for g, (goff, gsz) in enumerate(GROUPS):
                nc.vector.tensor_scalar(smt[:, :gsz], psS[g][:],
                                        invd[:, 0:1], None, op0=ALU.mult)
                nc.vector.tensor_scalar(sm1r[:, goff:goff + gsz],
                                        smt[:, :gsz], 1.0, None,
                                        op0=ALU.subtract)
            for g, (goff, gsz) in enumerate(GROUPS):
                psB = s5pp.tile([128, gsz], F32, tag=f"B{g}")
                nc.tensor.matmul(psB[:], ones1r[:], sm1r[:, goff:goff + gsz],
                                 start=True, stop=True)
                nc.vector.tensor_copy(spb[:, goff:goff + gsz], psB[:])

        # ---------- stage 6: A = xm @ w^T, epilogue ----------
        with tc.tile_pool(name="s6_sbuf", bufs=3) as s6p, \
             tc.tile_pool(name="s6_psum", bufs=2, space="PSUM") as s6pp, \
             tc.tile_pool(name="s6A_psum", bufs=1, space="PSUM") as s6ap:
            psA = {}
            for m in range(2):
                for g, (goff, gsz) in enumerate(GROUPS):
                    psA[(m, g)] = s6ap.tile([128, gsz], F32, tag=f"A{m}{g}", name=f"psA{m}{g}")
            for k in range(KC):
                xn2 = s6p.tile([128, 256], F32, tag="xn2", bufs=2)
                for m in range(2):
                    nc.sync.dma_start(
                        xn2[:, m * 128:(m + 1) * 128],
                        x[m * 128:(m + 1) * 128, k * 128:(k + 1) * 128])
                xtp2 = s6pp.tile([128, 256], F32, tag="xtp2")
                for m in range(2):
                    nc.tensor.transpose(xtp2[:, m * 128:(m + 1) * 128],
                                        xn2[:, m * 128:(m + 1) * 128],
                                        ident[:])
                xa = s6p.tile([128, 256], F32R, tag="xa", bufs=2)
                nc.scalar.copy(xa[:], xtp2[:])
                xm = s6p.tile([128, 256], F32R, tag="xm", bufs=2)
                nc.vector.tensor_scalar(xm[:], xa[:], mf_all[:, k:k + 1],
                                        None, op0=ALU.mult)
                for m in range(2):
                    for g, (goff, gsz) in enumerate(GROUPS):
                        nc.tensor.matmul(
                            psA[(m, g)][:],
                            xm[:, m * 128:(m + 1) * 128],
                            wt_all[:, k * OSH + goff:k * OSH + goff + gsz],
                            start=(k == 0), stop=(k == KC - 1))
            for m in range(2):
                for g, (goff, gsz) in enumerate(GROUPS):
                    tmpe = s6p.tile([128, 512], F32, tag="tmpe", name="tmpe", bufs=2)
                    ycb = s6p.tile([128, 512], F32, tag="ycb", name="ycb", bufs=2)
                    nc.sync.dma_start(ycb[:, :gsz], y_tmp[m][:, goff:goff + gsz])
                    nc.vector.tensor_mul(tmpe[:, :gsz], psA[(m, g)][:],
                                         spb[:, goff:goff + gsz])
                    nc.vector.tensor_add(tmpe[:, :gsz], tmpe[:, :gsz],
                                         ycb[:, :gsz])
                    nc.sync.dma_start(y[m * 128:(m + 1) * 128, goff:goff + gsz],
                                      tmpe[:, :gsz])

    nc.compile()
    return nc


_NC_CACHE = None


def _get_nc():
    global _NC_CACHE
    if _NC_CACHE is None:
        _NC_CACHE = build()
    return _NC_CACHE


def _make_inputs(x, weight, bias):
    xf = np.ascontiguousarray(x.reshape(TOK, IN), dtype=np.float32)
    in_maps = []
    for c in range(N_CORES):
        sel = np.zeros((4, 128, KC), dtype=np.float32)
        for tt in range(4):
            sel[tt, :, 4 * c + tt] = 1.0
        in_maps.append({
            "w": np.ascontiguousarray(weight[c * OSH:(c + 1) * OSH, :],
                                      dtype=np.float32),
            "x": xf,
            "bias": np.ascontiguousarray(
                bias[c * OSH:(c + 1) * OSH].reshape(1, OSH),
                dtype=np.float32),
            "sel": sel,
        })
    return in_maps


def kernel(x, weight, bias):
    x = np.asarray(x)
    weight = np.asarray(weight)
    bias = np.asarray(bias)
    nc = _get_nc()
    in_maps = _make_inputs(x, weight, bias)
    res = run_bass_kernel_spmd(nc, in_maps, list(range(N_CORES)))
    y = np.concatenate([res.results[c]["y"] for c in range(N_CORES)], axis=1)
    return y.reshape(B, S, OUT).astype(np.float32)


if __name__ == "__main__":
    rng = np.random.default_rng(0)
    x = rng.standard_normal((B, S, IN), dtype=np.float32)
    w = (rng.standard_normal((OUT, IN), dtype=np.float32) * 0.02).astype(np.float32)
    b = (rng.standard_normal((OUT,), dtype=np.float32) * 0.02).astype(np.float32)
    out = kernel(x=x, weight=w, bias=b)
    print("kernel output", out.shape, out.dtype)
